# revision 26
# baseline (speedup 1.0000x reference)
"""GCN layer (gather -> mean-aggregate -> linear -> relu) on 8 TRN2 NeuronCores.

Strategy (v3: fused gather instructions):
- Nodes/outputs sharded by destination across 8 cores (12500 dsts each);
  edges partitioned by dst core. h and the 64x64 weight replicated.
- out = relu(mask * (mean_agg(h) @ W.T + b)): the linear commutes with the
  mean, so cores gather raw h rows (bf16, padded to 128 cols = 256B gather
  elems), segment-sum per dst via one-hot selection matmuls in bf16, then
  project per 128-dst block.
- v3: the per-(dst-block, src-group) gather instructions of v2 paid ~1.9us
  of SWDGE overhead each (GpSimd 96% busy). Now cells are padded to
  128 rows (chunk-aligned) and many cells are fused into one dma_gather of
  up to NMAX indices; one SWDGE queue per src-group (4 equal groups of
  25000 src nodes) with a RING-deep gbuf ring per queue. All chunks are
  fully written by the gather (pad idxs = 0), so no memsets; pad rows are
  killed by the one-hot (dv = -1).
- HW limits found empirically: a dma_gather must keep n/16+1 <= 128
  descriptors per DMA engine (n <= 2032; we use 1920) and needs
  single_packet=False above n=1024 (16KB per-engine packet limit).
- Remaining bottleneck: Q7 SWDGE descriptor generation, ~8.5ns/idx per
  queue-pair, ~4 queues concurrent => ~2.1ns/idx for ~250k idxs/core.
  The 4 gather queues' instructions execute concurrently on separate Q7
  CPU pairs (Pool engine dispatch is only ~84ns per instruction).
"""

import numpy as np
from contextlib import ExitStack

N_NODES = 100000
N_EDGES = 1600000
D = 64
NCORES = 8
NPC = N_NODES // NCORES          # dsts per core
NB = (NPC + 127) // 128          # dst blocks per core
GS = 25000                       # src group size (4 equal groups)
NG = (N_NODES + GS - 1) // GS    # src groups == queues
NMAX = 1920                      # max idxs per gather instr (ring: n/16+1 <= 128 descs/engine)
RING = 4                         # gbuf ring depth per queue
SELBUFS = 8                      # sel8 ring depth (granules of 8 chunks)
DMA_SCRATCH = 16384              # SWDGE descriptor carveout


def _round128(x):
    return (x + 127) & ~127


def _host_partition(edge_src, edge_dst):
    """Partition edges by (core, src-group, dst-block); build the shared
    static instruction plan plus per-core idx / dv / degree arrays."""
    core = edge_dst // NPC
    per_core = []
    counts = np.zeros((NCORES, NG, NB), np.int64)
    for c in range(NCORES):
        m = np.nonzero(core == c)[0]
        src_c = edge_src[m]
        dst_c = edge_dst[m] - c * NPC
        blk = dst_c >> 7
        grp = src_c // GS
        order = np.lexsort((src_c, blk, grp))  # (grp, blk, src) sort
        src_c = src_c[order]
        dst_c = dst_c[order]
        cnt = np.bincount(grp[order] * NB + blk[order],
                          minlength=NG * NB).reshape(NG, NB)
        counts[c] = cnt
        per_core.append((src_c, dst_c))

    caps = counts.max(axis=0)               # [NG, NB]
    caps[0] = np.maximum(caps[0], 1)        # every B has >= 1 chunk
    caps = _round128(caps)                  # 0 stays 0 elsewhere

    # --- static instruction plan: per queue g, fuse consecutive cells ---
    # instr: dict(g, k, cells=[(B, cap)], n, wofs, chunk threshold info)
    plan = [[] for _ in range(NG)]
    for g in range(NG):
        cur = []
        n = 0
        for B in range(NB):
            cap = int(caps[g, B])
            if cap == 0:
                continue
            if n + cap > NMAX and cur:
                plan[g].append((cur, n))
                cur = []
                n = 0
            cur.append((B, cap))
            n += cap
        if cur:
            plan[g].append((cur, n))

    # global idx layout: queue-major, instruction order; wofs in 16-col units
    wofs_of = {}
    wofs = 0
    for g in range(NG):
        for k, (cells, n) in enumerate(plan[g]):
            wofs_of[(g, k)] = wofs
            wofs += n // 16
    idx_w = wofs

    # per-cell location: instr index + row offset within instruction
    cell_loc = {}
    for g in range(NG):
        for k, (cells, n) in enumerate(plan[g]):
            r = 0
            for (B, cap) in cells:
                cell_loc[(g, B)] = (k, r)
                r += cap

    # chunk bookkeeping in consumption (B-major) order
    nch = caps // 128                        # [NG, NB]
    blk_chunks = nch.sum(axis=0)             # [NB]
    blk_end = np.cumsum(blk_chunks)
    tot_chunks = int(blk_end[-1])
    # pos_end(B, g): 1-based count of global chunks consumed once cell
    # (g, B) is done
    pos_end = np.zeros((NG, NB), np.int64)
    run = 0
    for B in range(NB):
        base = 0 if B == 0 else int(blk_end[B - 1])
        run = base
        for g in range(NG):
            run += int(nch[g, B])
            pos_end[g, B] = run
    # per-instruction consumption threshold (for the gbuf ring wait)
    instr_thresh = [[int(pos_end[g, cells[-1][0]]) for (cells, n) in plan[g]]
                    for g in range(NG)]
    # chunk column offset of cell (g, B) in the dv/sel stream
    cell_chunk_col = {}
    for B in range(NB):
        col = 0 if B == 0 else int(blk_end[B - 1])
        for g in range(NG):
            if nch[g, B]:
                cell_chunk_col[(g, B)] = col
                col += int(nch[g, B])

    nch8 = (tot_chunks + 7) // 8

    idx_arrs = []
    dv_arrs = []
    deg_arrs = []
    for c in range(NCORES):
        src_c, dst_c = per_core[c]
        deg = np.bincount(dst_c, minlength=NB * 128).astype(np.float32)
        rdeg = 1.0 / np.maximum(deg, 1.0)
        ind = np.minimum(deg, 1.0)
        deg_arrs.append((rdeg.reshape(NB, 128).T.copy(),
                         ind.reshape(NB, 128).T.copy()))
        cnt = counts[c]                      # [NG, NB]
        cell_starts = np.zeros(NG * NB + 1, np.int64)
        np.cumsum(cnt.reshape(-1), out=cell_starts[1:])

        idx16 = np.zeros((128, idx_w), np.int16)
        dv = np.full((128, nch8 * 8), -1.0, np.float32)
        for g in range(NG):
            for k, (cells, n) in enumerate(plan[g]):
                flat = np.zeros(n, np.int16)
                r = 0
                for (B, cap) in cells:
                    ci = g * NB + B
                    kk = int(cnt[g, B])
                    s0 = int(cell_starts[ci])
                    if kk > 0:
                        flat[r:r + kk] = (src_c[s0:s0 + kk] - g * GS
                                          ).astype(np.int16)
                        e = np.arange(kk)
                        col0 = cell_chunk_col[(g, B)]
                        dv[e % 128, col0 + e // 128] = (
                            dst_c[s0:s0 + kk] & 127).astype(np.float32)
                    r += cap
                w0 = wofs_of[(g, k)]
                idx16[:, w0:w0 + n // 16] = np.tile(
                    flat.reshape(n // 16, 16).T, (8, 1))
        idx_arrs.append(idx16)
        dv_arrs.append(dv)

    meta = dict(plan=plan, wofs_of=wofs_of, cell_loc=cell_loc, nch=nch,
                blk_chunks=blk_chunks, blk_end=blk_end,
                tot_chunks=tot_chunks, nch8=nch8, idx_w=idx_w,
                instr_thresh=instr_thresh, caps=caps)
    return meta, idx_arrs, dv_arrs, deg_arrs


def _build_nc(meta):
    import concourse.bacc as bacc
    import concourse.mybir as mybir
    from concourse.library_config import mlp
    from concourse._compat import get_trn_type

    f32 = mybir.dt.float32
    bf16 = mybir.dt.bfloat16
    i16 = mybir.dt.int16

    plan = meta["plan"]
    wofs_of = meta["wofs_of"]
    cell_loc = meta["cell_loc"]
    nch = meta["nch"]
    blk_end = meta["blk_end"]
    tot_chunks = meta["tot_chunks"]
    nch8 = meta["nch8"]
    idx_w = meta["idx_w"]
    instr_thresh = meta["instr_thresh"]

    nc = bacc.Bacc(get_trn_type() or "TRN2", debug=True, num_swdge_queues=4,
                   dynamic_dma_scratch_size=DMA_SCRATCH)
    h_d = nc.declare_dram_parameter("h", [N_NODES, 128], bf16, isOutput=False)
    idx_d = nc.declare_dram_parameter("idx", [128, idx_w], i16, isOutput=False)
    dv_d = nc.declare_dram_parameter("dv", [128, nch8 * 8], bf16, isOutput=False)
    cst_d = nc.declare_dram_parameter("cst", [128, 256], bf16, isOutput=False)
    wa_d = nc.declare_dram_parameter("wa", [65, D], bf16, isOutput=False)
    rdeg_d = nc.declare_dram_parameter("rdeg", [128, NB], f32, isOutput=False)
    ind_d = nc.declare_dram_parameter("ind", [128, NB], bf16, isOutput=False)
    out_d = nc.declare_dram_parameter("out", [NB * 128, D], f32, isOutput=True)

    # per-queue idx spans (16-col units) for split input loads
    qspan = []
    for g in range(NG):
        w0 = wofs_of[(g, 0)]
        w1 = wofs_of[(g, len(plan[g]) - 1)] + plan[g][-1][1] // 16
        qspan.append((w0, w1))

    with ExitStack() as st:
        e = st.enter_context
        idx_sb = e(nc.sbuf_tensor("idx_sb", [128, idx_w], i16))
        dv_sb = e(nc.sbuf_tensor("dv_sb", [128, nch8 * 8], bf16))
        cst_sb = e(nc.sbuf_tensor("cst_sb", [128, 256], bf16))
        wa_sb = e(nc.sbuf_tensor("wa_sb", [65, D], bf16))
        rdeg_sb = e(nc.sbuf_tensor("rdeg_sb", [128, NB], f32))
        ind_sb = e(nc.sbuf_tensor("ind_sb", [128, NB], bf16))
        gbuf = [[e(nc.sbuf_tensor(f"gbuf{g}_{r}", [128, NMAX], bf16))
                 for r in range(RING)] for g in range(NG)]
        sel8 = [e(nc.sbuf_tensor(f"sel8_{i}", [128, 8 * 128], bf16))
                for i in range(SELBUFS)]
        agg = [e(nc.sbuf_tensor(f"agg{i}", [128, 65], bf16)) for i in range(2)]
        aggT = [e(nc.sbuf_tensor(f"aggT{i}", [65, 128], bf16)) for i in range(2)]
        otile = [e(nc.sbuf_tensor(f"otile{i}", [128, D], f32)) for i in range(2)]

        acc = [e(nc.psum_tensor(f"acc{i}", [128, D], f32)) for i in range(2)]
        pt1 = [e(nc.psum_tensor(f"pt1_{i}", [65, 128], bf16)) for i in range(2)]
        pmw = [e(nc.psum_tensor(f"pmw{i}", [128, D], f32)) for i in range(2)]

        in_s = e(nc.semaphore("in_s"))
        idx_s = [e(nc.semaphore(f"idx_s{i}")) for i in range(NG)]
        g_s = [e(nc.semaphore(f"g_s{i}")) for i in range(NG)]
        pe_s = e(nc.semaphore("pe_s"))
        sel_s = e(nc.semaphore("sel_s"))
        dep_s = e(nc.semaphore("dep_s"))
        pt1_s = e(nc.semaphore("pt1_s"))
        dt1_s = e(nc.semaphore("dt1_s"))
        pmw_s = e(nc.semaphore("pmw_s"))
        act_s = e(nc.semaphore("act_s"))
        out_s = e(nc.semaphore("out_s"))
        block = e(nc.Block())

        iota_ap = lambda: cst_sb[:, 0:128]
        ident_ap = lambda: cst_sb[:, 128:256]

        # input loads: idx span g increments g_s[g] (so queue g's first
        # gather waits only on its own span); the 5 small loads share in_s.
        # DMA completions are NOT ordered, so consumers wait for ALL of in_s.
        IN_ALL = 80

        @block.gpsimd
        def _(eng):
            eng.load_library(mlp)
            for g in range(NG):
                w0, w1 = qspan[g]
                eng.dma_start(out=idx_sb[:, w0:w1],
                              in_=idx_d[:, w0:w1]).then_inc(idx_s[g], 16)
            eng.dma_start(out=dv_sb[:], in_=dv_d[:]).then_inc(in_s, 16)
            eng.dma_start(out=cst_sb[:], in_=cst_d[:]).then_inc(in_s, 16)
            eng.dma_start(out=wa_sb[:], in_=wa_d[:]).then_inc(in_s, 16)
            eng.dma_start(out=rdeg_sb[:], in_=rdeg_d[:]).then_inc(in_s, 16)
            eng.dma_start(out=ind_sb[:], in_=ind_d[:]).then_inc(in_s, 16)

            rounds = max(len(plan[g]) for g in range(NG))
            waited_idx = [False] * NG
            for k in range(rounds):
                for g in range(NG):
                    if k >= len(plan[g]):
                        continue
                    if not waited_idx[g]:
                        eng.wait_ge(idx_s[g], 16)
                        waited_idx[g] = True
                    if k >= RING:
                        eng.wait_ge(pe_s, instr_thresh[g][k - RING])
                    cells, n = plan[g][k]
                    kb = n // 128
                    w0 = wofs_of[(g, k)]
                    eng.dma_gather(
                        out_ap=gbuf[g][k % RING][:, : kb * 128].rearrange(
                            "p (k d) -> p k d", d=128
                        ),
                        in_ap=h_d[g * GS: (g + 1) * GS, :],
                        idxs_ap=idx_sb[:, w0: w0 + n // 16],
                        num_idxs=n,
                        num_idxs_reg=n,
                        elem_size=128,
                        single_packet=False,
                        queue_num=g,
                    ).then_inc(g_s[g], 16)

        @block.tensor
        def _(eng):
            eng.wait_ge(in_s, IN_ALL)

            def pe_pt1(Bp):
                p = Bp % 2
                eng.wait_ge(dep_s, Bp + 1)
                if Bp >= 2:
                    eng.wait_ge(dt1_s, Bp - 1)
                eng.matmul(
                    out=pt1[p][:], lhsT=agg[p][:], rhs=ident_ap(),
                    is_transpose=True,
                ).then_inc(pt1_s, 1)

            def pe_pmw(Bp):
                p = Bp % 2
                eng.wait_ge(dt1_s, Bp + 1)
                if Bp >= 2:
                    eng.wait_ge(act_s, Bp - 1)
                eng.matmul(
                    out=pmw[p][:], lhsT=aggT[p][:], rhs=wa_sb[:],
                    start=True, stop=True,
                ).then_inc(pmw_s, 1)

            kchunk = 0
            waited_instr = [-1] * NG  # last instr index waited per queue
            for B in range(NB):
                cb = int(sum(nch[g, B] for g in range(NG)))
                j = 0
                for g in range(NG):
                    ncell = int(nch[g, B])
                    if ncell == 0:
                        continue
                    k_in, r0 = cell_loc[(g, B)]
                    # consume instr k only once k+1 is also complete: the
                    # per-engine completion sums can transiently reach
                    # 16*(k+1) with a lagging engine still mid-k when
                    # several instructions are in flight; the +1 slack
                    # (capped at the queue's last instruction) closes that
                    # window at no pipeline cost.
                    if waited_instr[g] < k_in:
                        eng.wait_ge(g_s[g], 16 * min(k_in + 2, len(plan[g])))
                        waited_instr[g] = k_in
                    for jc in range(ncell):
                        if kchunk % 8 == 0:
                            eng.wait_ge(sel_s, kchunk // 8 + 1)
                        if j == 0 and B >= 2:
                            eng.wait_ge(dep_s, B - 1)
                        cofs = r0 + jc * 128
                        eng.matmul(
                            out=acc[B % 2][:],
                            lhsT=sel8[(kchunk // 8) % SELBUFS][
                                :, (kchunk % 8) * 128: (kchunk % 8) * 128 + 128
                            ],
                            rhs=gbuf[g][k_in % RING][:, cofs: cofs + 64],
                            start=(j == 0), stop=(j == cb - 1),
                        ).then_inc(pe_s, 1)
                        kchunk += 1
                        j += 1
                if B >= 1:
                    pe_pt1(B - 1)
                if B >= 2:
                    pe_pmw(B - 2)
            pe_pt1(NB - 1)
            pe_pmw(NB - 2)
            pe_pmw(NB - 1)

        @block.vector
        def _(eng):
            import concourse.mybir as mb
            eng.wait_ge(in_s, IN_ALL)

            def dve_ep(Bp):
                p = Bp % 2
                eng.wait_ge(pe_s, int(blk_end[Bp]))
                if Bp >= 2:
                    eng.wait_ge(pt1_s, Bp - 1)
                eng.tensor_scalar(
                    out=agg[p][:, 0:64], in0=acc[p][:],
                    scalar1=rdeg_sb[:, Bp: Bp + 1], scalar2=None,
                    op0=mb.AluOpType.mult,
                )
                eng.tensor_copy(
                    out=agg[p][:, 64:65], in_=ind_sb[:, Bp: Bp + 1]
                ).then_inc(dep_s, 1)

            gi = 0
            emitted = 0
            for B in range(NB):
                while emitted < int(blk_end[B]):
                    if gi >= SELBUFS:
                        eng.wait_ge(pe_s, 8 * (gi - (SELBUFS - 1)))
                    eng.tensor_tensor(
                        out=sel8[gi % SELBUFS][:].rearrange(
                            "p (c f) -> p c f", f=128),
                        in0=dv_sb[:, gi * 8: gi * 8 + 8].to_broadcast(
                            [128, 8, 128]),
                        in1=iota_ap().rearrange(
                            "p (o f) -> p o f", o=1).to_broadcast(
                            [128, 8, 128]),
                        op=mb.AluOpType.is_equal,
                    ).then_inc(sel_s, 1)
                    gi += 1
                    emitted += 8
                if B >= 1:
                    dve_ep(B - 1)
            dve_ep(NB - 1)

        @block.scalar
        def _(eng):
            import concourse.mybir as mb
            eng.wait_ge(in_s, IN_ALL)

            def relu_step(Bp):
                p = Bp % 2
                eng.wait_ge(pmw_s, Bp + 1)
                if Bp >= 2:
                    eng.wait_ge(out_s, 16 * (Bp - 1))
                eng.activation(
                    out=otile[p][:], in_=pmw[p][:],
                    func=mb.ActivationFunctionType.Relu,
                ).then_inc(act_s, 1)

            for B in range(NB):
                eng.wait_ge(pt1_s, B + 1)
                if B >= 2:
                    eng.wait_ge(pmw_s, B - 1)
                eng.activation(
                    out=aggT[B % 2][:], in_=pt1[B % 2][:],
                    func=mb.ActivationFunctionType.Copy,
                ).then_inc(dt1_s, 1)
                if B >= 2:
                    relu_step(B - 2)
            relu_step(NB - 2)
            relu_step(NB - 1)

        @block.sync
        def _(eng):
            for B in range(NB):
                eng.wait_ge(act_s, B + 1)
                eng.dma_start(
                    out=out_d[B * 128: (B + 1) * 128, :], in_=otile[B % 2][:]
                ).then_inc(out_s, 16)
            eng.wait_ge(out_s, 16 * NB)

    nc.compile()
    return nc


def _host_inputs(h, W, b, idx_arrs, dv_arrs, deg_arrs):
    import concourse.mybir as mybir
    bf16 = mybir.dt.np(mybir.dt.bfloat16)

    h_pad = np.zeros((N_NODES, 128), dtype=bf16)
    h_pad[:, 0:64] = h.astype(bf16)

    cst = np.zeros((128, 256), np.float32)
    cst[:, 0:128] = np.arange(128, dtype=np.float32)[None, :]
    cst[:, 128:256] = np.eye(128, dtype=np.float32)
    cst = cst.astype(bf16)

    wa = np.concatenate(
        [W.T.astype(np.float32), b.astype(np.float32)[None, :]], axis=0
    ).astype(bf16)

    in_maps = []
    for c in range(NCORES):
        in_maps.append({
            "h": h_pad,
            "idx": idx_arrs[c],
            "dv": dv_arrs[c].astype(bf16),
            "cst": cst,
            "wa": wa,
            "rdeg": deg_arrs[c][0],
            "ind": deg_arrs[c][1].astype(bf16),
        })
    return in_maps


def kernel(h, edge_src, edge_dst, W, b):
    h = np.asarray(h, np.float32)
    edge_src = np.asarray(edge_src, np.int32)
    edge_dst = np.asarray(edge_dst, np.int32)
    W = np.asarray(W, np.float32)
    b = np.asarray(b, np.float32)

    from concourse.bass_utils import run_bass_kernel_spmd

    meta, idx_arrs, dv_arrs, deg_arrs = _host_partition(edge_src, edge_dst)
    nc = _build_nc(meta)
    in_maps = _host_inputs(h, W, b, idx_arrs, dv_arrs, deg_arrs)
    res = run_bass_kernel_spmd(nc, in_maps, list(range(NCORES)))
    out = np.concatenate(
        [res.results[c]["out"][:NPC] for c in range(NCORES)], axis=0
    )
    return out.astype(np.float32)


# revision 28
# speedup vs baseline: 1.0227x; 1.0227x over previous
"""GCN layer (gather -> mean-aggregate -> linear -> relu) on 8 TRN2 NeuronCores.

Strategy (v3: fused gather instructions):
- Nodes/outputs sharded by destination across 8 cores (12500 dsts each);
  edges partitioned by dst core. h and the 64x64 weight replicated.
- out = relu(mask * (mean_agg(h) @ W.T + b)): the linear commutes with the
  mean, so cores gather raw h rows (bf16, padded to 128 cols = 256B gather
  elems), segment-sum per dst via one-hot selection matmuls in bf16, then
  project per 128-dst block.
- v3: the per-(dst-block, src-group) gather instructions of v2 paid ~1.9us
  of SWDGE overhead each (GpSimd 96% busy). Now cells are padded to
  128 rows (chunk-aligned) and many cells are fused into one dma_gather of
  up to NMAX indices; one SWDGE queue per src-group (4 equal groups of
  25000 src nodes) with a RING-deep gbuf ring per queue. All chunks are
  fully written by the gather (pad idxs = 0), so no memsets; pad rows are
  killed by the one-hot (dv = -1).
- HW limits found empirically: a dma_gather must keep n/16+1 <= 128
  descriptors per DMA engine (n <= 2032; we use 1920) and needs
  single_packet=False above n=1024 (16KB per-engine packet limit).
- Remaining bottleneck: Q7 SWDGE descriptor generation, ~8.5ns/idx per
  queue-pair, ~4 queues concurrent => ~2.1ns/idx for ~250k idxs/core.
  The 4 gather queues' instructions execute concurrently on separate Q7
  CPU pairs (Pool engine dispatch is only ~84ns per instruction).
"""

import numpy as np
from contextlib import ExitStack

N_NODES = 100000
N_EDGES = 1600000
D = 64
NCORES = 8
NPC = N_NODES // NCORES          # dsts per core
NB = (NPC + 127) // 128          # dst blocks per core
GS = 25000                       # src group size (4 equal groups)
NG = (N_NODES + GS - 1) // GS    # src groups == queues
NMAX = 1920                      # max idxs per gather instr (ring: n/16+1 <= 128 descs/engine)
RING = 5                         # gbuf ring depth per queue (+1 compensates the PE consume slack)
SELBUFS = 8                      # sel8 ring depth (granules of 8 chunks)
DMA_SCRATCH = 16384              # SWDGE descriptor carveout


def _round128(x):
    return (x + 127) & ~127


def _host_partition(edge_src, edge_dst):
    """Partition edges by (core, src-group, dst-block); build the shared
    static instruction plan plus per-core idx / dv / degree arrays."""
    core = edge_dst // NPC
    per_core = []
    counts = np.zeros((NCORES, NG, NB), np.int64)
    for c in range(NCORES):
        m = np.nonzero(core == c)[0]
        src_c = edge_src[m]
        dst_c = edge_dst[m] - c * NPC
        blk = dst_c >> 7
        grp = src_c // GS
        order = np.lexsort((src_c, blk, grp))  # (grp, blk, src) sort
        src_c = src_c[order]
        dst_c = dst_c[order]
        cnt = np.bincount(grp[order] * NB + blk[order],
                          minlength=NG * NB).reshape(NG, NB)
        counts[c] = cnt
        per_core.append((src_c, dst_c))

    caps = counts.max(axis=0)               # [NG, NB]
    caps[0] = np.maximum(caps[0], 1)        # every B has >= 1 chunk
    caps = _round128(caps)                  # 0 stays 0 elsewhere

    # --- static instruction plan: per queue g, fuse consecutive cells ---
    # instr: dict(g, k, cells=[(B, cap)], n, wofs, chunk threshold info)
    plan = [[] for _ in range(NG)]
    for g in range(NG):
        cur = []
        n = 0
        for B in range(NB):
            cap = int(caps[g, B])
            if cap == 0:
                continue
            if n + cap > NMAX and cur:
                plan[g].append((cur, n))
                cur = []
                n = 0
            cur.append((B, cap))
            n += cap
        if cur:
            plan[g].append((cur, n))

    # global idx layout: queue-major, instruction order; wofs in 16-col units
    wofs_of = {}
    wofs = 0
    for g in range(NG):
        for k, (cells, n) in enumerate(plan[g]):
            wofs_of[(g, k)] = wofs
            wofs += n // 16
    idx_w = wofs

    # per-cell location: instr index + row offset within instruction
    cell_loc = {}
    for g in range(NG):
        for k, (cells, n) in enumerate(plan[g]):
            r = 0
            for (B, cap) in cells:
                cell_loc[(g, B)] = (k, r)
                r += cap

    # chunk bookkeeping in consumption (B-major) order
    nch = caps // 128                        # [NG, NB]
    blk_chunks = nch.sum(axis=0)             # [NB]
    blk_end = np.cumsum(blk_chunks)
    tot_chunks = int(blk_end[-1])
    # pos_end(B, g): 1-based count of global chunks consumed once cell
    # (g, B) is done
    pos_end = np.zeros((NG, NB), np.int64)
    run = 0
    for B in range(NB):
        base = 0 if B == 0 else int(blk_end[B - 1])
        run = base
        for g in range(NG):
            run += int(nch[g, B])
            pos_end[g, B] = run
    # per-instruction consumption threshold (for the gbuf ring wait)
    instr_thresh = [[int(pos_end[g, cells[-1][0]]) for (cells, n) in plan[g]]
                    for g in range(NG)]
    # chunk column offset of cell (g, B) in the dv/sel stream
    cell_chunk_col = {}
    for B in range(NB):
        col = 0 if B == 0 else int(blk_end[B - 1])
        for g in range(NG):
            if nch[g, B]:
                cell_chunk_col[(g, B)] = col
                col += int(nch[g, B])

    nch8 = (tot_chunks + 7) // 8

    idx_arrs = []
    dv_arrs = []
    deg_arrs = []
    for c in range(NCORES):
        src_c, dst_c = per_core[c]
        deg = np.bincount(dst_c, minlength=NB * 128).astype(np.float32)
        rdeg = 1.0 / np.maximum(deg, 1.0)
        ind = np.minimum(deg, 1.0)
        deg_arrs.append((rdeg.reshape(NB, 128).T.copy(),
                         ind.reshape(NB, 128).T.copy()))
        cnt = counts[c]                      # [NG, NB]
        cell_starts = np.zeros(NG * NB + 1, np.int64)
        np.cumsum(cnt.reshape(-1), out=cell_starts[1:])

        idx16 = np.zeros((128, idx_w), np.int16)
        dv = np.full((128, nch8 * 8), -1.0, np.float32)
        for g in range(NG):
            for k, (cells, n) in enumerate(plan[g]):
                flat = np.zeros(n, np.int16)
                r = 0
                for (B, cap) in cells:
                    ci = g * NB + B
                    kk = int(cnt[g, B])
                    s0 = int(cell_starts[ci])
                    if kk > 0:
                        flat[r:r + kk] = (src_c[s0:s0 + kk] - g * GS
                                          ).astype(np.int16)
                        e = np.arange(kk)
                        col0 = cell_chunk_col[(g, B)]
                        dv[e % 128, col0 + e // 128] = (
                            dst_c[s0:s0 + kk] & 127).astype(np.float32)
                    r += cap
                w0 = wofs_of[(g, k)]
                idx16[:, w0:w0 + n // 16] = np.tile(
                    flat.reshape(n // 16, 16).T, (8, 1))
        idx_arrs.append(idx16)
        dv_arrs.append(dv)

    meta = dict(plan=plan, wofs_of=wofs_of, cell_loc=cell_loc, nch=nch,
                blk_chunks=blk_chunks, blk_end=blk_end,
                tot_chunks=tot_chunks, nch8=nch8, idx_w=idx_w,
                instr_thresh=instr_thresh, caps=caps)
    return meta, idx_arrs, dv_arrs, deg_arrs


def _build_nc(meta):
    import concourse.bacc as bacc
    import concourse.mybir as mybir
    from concourse.library_config import mlp
    from concourse._compat import get_trn_type

    f32 = mybir.dt.float32
    bf16 = mybir.dt.bfloat16
    i16 = mybir.dt.int16

    plan = meta["plan"]
    wofs_of = meta["wofs_of"]
    cell_loc = meta["cell_loc"]
    nch = meta["nch"]
    blk_end = meta["blk_end"]
    tot_chunks = meta["tot_chunks"]
    nch8 = meta["nch8"]
    idx_w = meta["idx_w"]
    instr_thresh = meta["instr_thresh"]

    nc = bacc.Bacc(get_trn_type() or "TRN2", debug=True, num_swdge_queues=4,
                   dynamic_dma_scratch_size=DMA_SCRATCH)
    h_d = nc.declare_dram_parameter("h", [N_NODES, 128], bf16, isOutput=False)
    idx_d = nc.declare_dram_parameter("idx", [128, idx_w], i16, isOutput=False)
    dv_d = nc.declare_dram_parameter("dv", [128, nch8 * 8], bf16, isOutput=False)
    cst_d = nc.declare_dram_parameter("cst", [128, 256], bf16, isOutput=False)
    wa_d = nc.declare_dram_parameter("wa", [65, D], bf16, isOutput=False)
    rdeg_d = nc.declare_dram_parameter("rdeg", [128, NB], f32, isOutput=False)
    ind_d = nc.declare_dram_parameter("ind", [128, NB], bf16, isOutput=False)
    out_d = nc.declare_dram_parameter("out", [NB * 128, D], f32, isOutput=True)

    # per-queue idx spans (16-col units) for split input loads
    qspan = []
    for g in range(NG):
        w0 = wofs_of[(g, 0)]
        w1 = wofs_of[(g, len(plan[g]) - 1)] + plan[g][-1][1] // 16
        qspan.append((w0, w1))

    with ExitStack() as st:
        e = st.enter_context
        idx_sb = e(nc.sbuf_tensor("idx_sb", [128, idx_w], i16))
        dv_sb = e(nc.sbuf_tensor("dv_sb", [128, nch8 * 8], bf16))
        cst_sb = e(nc.sbuf_tensor("cst_sb", [128, 256], bf16))
        wa_sb = e(nc.sbuf_tensor("wa_sb", [65, D], bf16))
        rdeg_sb = e(nc.sbuf_tensor("rdeg_sb", [128, NB], f32))
        ind_sb = e(nc.sbuf_tensor("ind_sb", [128, NB], bf16))
        gbuf = [[e(nc.sbuf_tensor(f"gbuf{g}_{r}", [128, NMAX], bf16))
                 for r in range(RING)] for g in range(NG)]
        sel8 = [e(nc.sbuf_tensor(f"sel8_{i}", [128, 8 * 128], bf16))
                for i in range(SELBUFS)]
        agg = [e(nc.sbuf_tensor(f"agg{i}", [128, 65], bf16)) for i in range(2)]
        aggT = [e(nc.sbuf_tensor(f"aggT{i}", [65, 128], bf16)) for i in range(2)]
        otile = [e(nc.sbuf_tensor(f"otile{i}", [128, D], f32)) for i in range(2)]

        acc = [e(nc.psum_tensor(f"acc{i}", [128, D], f32)) for i in range(2)]
        pt1 = [e(nc.psum_tensor(f"pt1_{i}", [65, 128], bf16)) for i in range(2)]
        pmw = [e(nc.psum_tensor(f"pmw{i}", [128, D], f32)) for i in range(2)]

        in_s = e(nc.semaphore("in_s"))
        idx_s = [e(nc.semaphore(f"idx_s{i}")) for i in range(NG)]
        g_s = [e(nc.semaphore(f"g_s{i}")) for i in range(NG)]
        pe_s = e(nc.semaphore("pe_s"))
        sel_s = e(nc.semaphore("sel_s"))
        dep_s = e(nc.semaphore("dep_s"))
        pt1_s = e(nc.semaphore("pt1_s"))
        dt1_s = e(nc.semaphore("dt1_s"))
        pmw_s = e(nc.semaphore("pmw_s"))
        act_s = e(nc.semaphore("act_s"))
        out_s = e(nc.semaphore("out_s"))
        block = e(nc.Block())

        iota_ap = lambda: cst_sb[:, 0:128]
        ident_ap = lambda: cst_sb[:, 128:256]

        # input loads: idx span g increments g_s[g] (so queue g's first
        # gather waits only on its own span); the 5 small loads share in_s.
        # DMA completions are NOT ordered, so consumers wait for ALL of in_s.
        IN_ALL = 80

        @block.gpsimd
        def _(eng):
            eng.load_library(mlp)
            # small tensors first: DVE/PE/Scalar wait on all of in_s, and
            # these ~5KB/partition loads finish long before the idx spans
            eng.dma_start(out=dv_sb[:], in_=dv_d[:]).then_inc(in_s, 16)
            eng.dma_start(out=cst_sb[:], in_=cst_d[:]).then_inc(in_s, 16)
            eng.dma_start(out=wa_sb[:], in_=wa_d[:]).then_inc(in_s, 16)
            eng.dma_start(out=rdeg_sb[:], in_=rdeg_d[:]).then_inc(in_s, 16)
            eng.dma_start(out=ind_sb[:], in_=ind_d[:]).then_inc(in_s, 16)
            for g in range(NG):
                w0, w1 = qspan[g]
                eng.dma_start(out=idx_sb[:, w0:w1],
                              in_=idx_d[:, w0:w1]).then_inc(idx_s[g], 16)

            rounds = max(len(plan[g]) for g in range(NG))
            waited_idx = [False] * NG
            for k in range(rounds):
                for g in range(NG):
                    if k >= len(plan[g]):
                        continue
                    if not waited_idx[g]:
                        eng.wait_ge(idx_s[g], 16)
                        waited_idx[g] = True
                    if k >= RING:
                        eng.wait_ge(pe_s, instr_thresh[g][k - RING])
                    cells, n = plan[g][k]
                    kb = n // 128
                    w0 = wofs_of[(g, k)]
                    eng.dma_gather(
                        out_ap=gbuf[g][k % RING][:, : kb * 128].rearrange(
                            "p (k d) -> p k d", d=128
                        ),
                        in_ap=h_d[g * GS: (g + 1) * GS, :],
                        idxs_ap=idx_sb[:, w0: w0 + n // 16],
                        num_idxs=n,
                        num_idxs_reg=n,
                        elem_size=128,
                        single_packet=False,
                        queue_num=g,
                    ).then_inc(g_s[g], 16)

        @block.tensor
        def _(eng):
            eng.wait_ge(in_s, IN_ALL)

            def pe_pt1(Bp):
                p = Bp % 2
                eng.wait_ge(dep_s, Bp + 1)
                if Bp >= 2:
                    eng.wait_ge(dt1_s, Bp - 1)
                eng.matmul(
                    out=pt1[p][:], lhsT=agg[p][:], rhs=ident_ap(),
                    is_transpose=True,
                ).then_inc(pt1_s, 1)

            def pe_pmw(Bp):
                p = Bp % 2
                eng.wait_ge(dt1_s, Bp + 1)
                if Bp >= 2:
                    eng.wait_ge(act_s, Bp - 1)
                eng.matmul(
                    out=pmw[p][:], lhsT=aggT[p][:], rhs=wa_sb[:],
                    start=True, stop=True,
                ).then_inc(pmw_s, 1)

            kchunk = 0
            waited_instr = [-1] * NG  # last instr index waited per queue
            for B in range(NB):
                cb = int(sum(nch[g, B] for g in range(NG)))
                j = 0
                for g in range(NG):
                    ncell = int(nch[g, B])
                    if ncell == 0:
                        continue
                    k_in, r0 = cell_loc[(g, B)]
                    # consume instr k only once k+1 is also complete: the
                    # per-engine completion sums can transiently reach
                    # 16*(k+1) with a lagging engine still mid-k when
                    # several instructions are in flight; the +1 slack
                    # (capped at the queue's last instruction) closes that
                    # window at no pipeline cost.
                    if waited_instr[g] < k_in:
                        eng.wait_ge(g_s[g], 16 * min(k_in + 2, len(plan[g])))
                        waited_instr[g] = k_in
                    for jc in range(ncell):
                        if kchunk % 8 == 0:
                            eng.wait_ge(sel_s, kchunk // 8 + 1)
                        if j == 0 and B >= 2:
                            eng.wait_ge(dep_s, B - 1)
                        cofs = r0 + jc * 128
                        eng.matmul(
                            out=acc[B % 2][:],
                            lhsT=sel8[(kchunk // 8) % SELBUFS][
                                :, (kchunk % 8) * 128: (kchunk % 8) * 128 + 128
                            ],
                            rhs=gbuf[g][k_in % RING][:, cofs: cofs + 64],
                            start=(j == 0), stop=(j == cb - 1),
                        ).then_inc(pe_s, 1)
                        kchunk += 1
                        j += 1
                if B >= 1:
                    pe_pt1(B - 1)
                if B >= 2:
                    pe_pmw(B - 2)
            pe_pt1(NB - 1)
            pe_pmw(NB - 2)
            pe_pmw(NB - 1)

        @block.vector
        def _(eng):
            import concourse.mybir as mb
            eng.wait_ge(in_s, IN_ALL)

            def dve_ep(Bp):
                p = Bp % 2
                eng.wait_ge(pe_s, int(blk_end[Bp]))
                if Bp >= 2:
                    eng.wait_ge(pt1_s, Bp - 1)
                eng.tensor_scalar(
                    out=agg[p][:, 0:64], in0=acc[p][:],
                    scalar1=rdeg_sb[:, Bp: Bp + 1], scalar2=None,
                    op0=mb.AluOpType.mult,
                )
                eng.tensor_copy(
                    out=agg[p][:, 64:65], in_=ind_sb[:, Bp: Bp + 1]
                ).then_inc(dep_s, 1)

            gi = 0
            emitted = 0
            for B in range(NB):
                while emitted < int(blk_end[B]):
                    if gi >= SELBUFS:
                        eng.wait_ge(pe_s, 8 * (gi - (SELBUFS - 1)))
                    eng.tensor_tensor(
                        out=sel8[gi % SELBUFS][:].rearrange(
                            "p (c f) -> p c f", f=128),
                        in0=dv_sb[:, gi * 8: gi * 8 + 8].to_broadcast(
                            [128, 8, 128]),
                        in1=iota_ap().rearrange(
                            "p (o f) -> p o f", o=1).to_broadcast(
                            [128, 8, 128]),
                        op=mb.AluOpType.is_equal,
                    ).then_inc(sel_s, 1)
                    gi += 1
                    emitted += 8
                if B >= 1:
                    dve_ep(B - 1)
            dve_ep(NB - 1)

        @block.scalar
        def _(eng):
            import concourse.mybir as mb
            eng.wait_ge(in_s, IN_ALL)

            def relu_step(Bp):
                p = Bp % 2
                eng.wait_ge(pmw_s, Bp + 1)
                if Bp >= 2:
                    eng.wait_ge(out_s, 16 * (Bp - 1))
                eng.activation(
                    out=otile[p][:], in_=pmw[p][:],
                    func=mb.ActivationFunctionType.Relu,
                ).then_inc(act_s, 1)

            for B in range(NB):
                eng.wait_ge(pt1_s, B + 1)
                if B >= 2:
                    eng.wait_ge(pmw_s, B - 1)
                eng.activation(
                    out=aggT[B % 2][:], in_=pt1[B % 2][:],
                    func=mb.ActivationFunctionType.Copy,
                ).then_inc(dt1_s, 1)
                if B >= 2:
                    relu_step(B - 2)
            relu_step(NB - 2)
            relu_step(NB - 1)

        @block.sync
        def _(eng):
            for B in range(NB):
                eng.wait_ge(act_s, B + 1)
                eng.dma_start(
                    out=out_d[B * 128: (B + 1) * 128, :], in_=otile[B % 2][:]
                ).then_inc(out_s, 16)
            eng.wait_ge(out_s, 16 * NB)

    nc.compile()
    return nc


def _host_inputs(h, W, b, idx_arrs, dv_arrs, deg_arrs):
    import concourse.mybir as mybir
    bf16 = mybir.dt.np(mybir.dt.bfloat16)

    h_pad = np.zeros((N_NODES, 128), dtype=bf16)
    h_pad[:, 0:64] = h.astype(bf16)

    cst = np.zeros((128, 256), np.float32)
    cst[:, 0:128] = np.arange(128, dtype=np.float32)[None, :]
    cst[:, 128:256] = np.eye(128, dtype=np.float32)
    cst = cst.astype(bf16)

    wa = np.concatenate(
        [W.T.astype(np.float32), b.astype(np.float32)[None, :]], axis=0
    ).astype(bf16)

    in_maps = []
    for c in range(NCORES):
        in_maps.append({
            "h": h_pad,
            "idx": idx_arrs[c],
            "dv": dv_arrs[c].astype(bf16),
            "cst": cst,
            "wa": wa,
            "rdeg": deg_arrs[c][0],
            "ind": deg_arrs[c][1].astype(bf16),
        })
    return in_maps


def kernel(h, edge_src, edge_dst, W, b):
    h = np.asarray(h, np.float32)
    edge_src = np.asarray(edge_src, np.int32)
    edge_dst = np.asarray(edge_dst, np.int32)
    W = np.asarray(W, np.float32)
    b = np.asarray(b, np.float32)

    from concourse.bass_utils import run_bass_kernel_spmd

    meta, idx_arrs, dv_arrs, deg_arrs = _host_partition(edge_src, edge_dst)
    nc = _build_nc(meta)
    in_maps = _host_inputs(h, W, b, idx_arrs, dv_arrs, deg_arrs)
    res = run_bass_kernel_spmd(nc, in_maps, list(range(NCORES)))
    out = np.concatenate(
        [res.results[c]["out"][:NPC] for c in range(NCORES)], axis=0
    )
    return out.astype(np.float32)
